# revision 20
# baseline (speedup 1.0000x reference)
"""Trainium2 Bass kernel for ClusterSeparationOptimizer.

Math (verified exactly vs reference):
  signed[i,n,j,h] = [x, y, 1] @ (A_i @ W[:, j, h])   (affine in the RAW point)
  mn = min_h signed,  nmx = -max_h signed            (over valid edges)
  v  = max(mn, nmx)
  viol = (v >= -EPS) * max(sigmoid(v), 0.5) * cluster_mask
  out  = sum viol (i!=j, hull_ok) + 0.1*|translations|^2 + |angles|^2

Geometric pruning (exact, not approximate): points of each cluster are
kd-sorted into 12 chunks of 128 with tight bboxes. A (chunk, hull j) pair
whose bboxes (with margin 1e-3 >> EPS) don't overlap contributes exactly 0
(every point is strictly outside hull j: some edge has s < -EPS and some
edge has s > +EPS, so v < -EPS and the term is gated to zero). Only ~8% of
pairs survive on typical data.

Device kernel (SPMD, one program, per-core data): T tiles, each tile =
(128 points, 3 hull-slots = 120 signed columns) via one K=3 fp32 matmul.
8 tiles share one 2-bank PSUM group; per group one tensor_reduce(min) and
one tensor_reduce(max, negate) over the edge axis (4D access pattern), a
batched tail (TT max, ACT sigmoid + relu gate, tensor_scalar, predicated
copy) into an SBUF strip, then one final reduce + ones-matmul -> scalar.
Hull-slot padding uses +-BIG poison columns (forces v=-BIG -> gated to 0).
Scalar partials are summed on host (the all-reduce).
"""

import numpy as np

C, N, H = 24, 1536, 40
NCORES = 8
PCHUNK = 128
NCHUNK = N // PCHUNK       # 12 chunks per cluster
SLOTS = 3                  # hull j-slots per tile
TW = SLOTS * H             # 120 free columns per tile
GRP = 8                    # tiles per PSUM group (8 * 128 cols * 4B = 2 banks)
SEP_W, T_PEN, R_PEN = 1.0, 0.1, 1.0
EPS = 1e-8
BIG = 1e30
MARGIN = 1e-2

_NC_CACHE = {}


def _transform64(x, med, ang, tr):
    c, s = np.cos(ang), np.sin(ang)
    xc = x[..., 0] - med[:, None, 0]
    yc = x[..., 1] - med[:, None, 1]
    px = c[:, None] * xc - s[:, None] * yc + (med[:, 0] + tr[:, 0])[:, None]
    py = s[:, None] * xc + c[:, None] * yc + (med[:, 1] + tr[:, 1])[:, None]
    return np.stack([px, py], -1)


def _host_coeffs(ph, med, ang, tr, hm):
    """G[i] = A_i @ W: (C, 3, C, H) float64; rows act on [x, y, 1]."""
    hulT = _transform64(ph, med, ang, tr)
    hx, hy = hulT[..., 0], hulT[..., 1]
    ex = np.roll(hx, -1, axis=1) - hx
    ey = np.roll(hy, -1, axis=1) - hy
    elen_raw = np.sqrt(ex * ex + ey * ey)
    elen = elen_raw + EPS
    evalid = elen_raw > 1e-6
    a = ex / elen
    b = -ey / elen
    d = -(ex * hy - ey * hx) / elen

    W = np.stack([b, a, d], axis=0)  # (3, C, H): coefficients on [px', py', 1]
    degenerate = np.zeros(C, bool)
    for j in range(C):
        inv = ~evalid[j]
        if inv.any():
            val = np.nonzero(evalid[j])[0]
            if len(val) > 0:
                W[:, j, inv] = W[:, j, val[-1]][:, None]
            else:
                # no valid edges: reference yields viol=1 (min over empty=inf)
                W[:, j, :] = np.array([0.0, 0.0, BIG])[:, None]
                degenerate[j] = True

    c, s = np.cos(ang), np.sin(ang)
    A = np.zeros((C, 3, 3))
    A[:, 0, 0] = c
    A[:, 0, 1] = s
    A[:, 1, 0] = -s
    A[:, 1, 1] = c
    A[:, 2, 0] = med[:, 0] + tr[:, 0] - c * med[:, 0] + s * med[:, 1]
    A[:, 2, 1] = med[:, 1] + tr[:, 1] - s * med[:, 0] - c * med[:, 1]
    A[:, 2, 2] = 1.0

    G = np.einsum("ikl,lm->ikm", A, W.reshape(3, C * H))
    return G.reshape(C, 3, C, H), hulT, degenerate


def _kd_chunks(p):
    """Split points into 12 chunks of 128 via 3x2x2 median splits."""
    def split(ids, parts):
        if parts == 1:
            return [ids]
        q = p[ids]
        dim = 0 if np.ptp(q[:, 0]) >= np.ptp(q[:, 1]) else 1
        order = ids[np.argsort(q[:, dim], kind="stable")]
        if parts % 3 == 0:
            k = len(order) // 3
            return (split(order[:k], parts // 3)
                    + split(order[k:2 * k], parts // 3)
                    + split(order[2 * k:], parts // 3))
        k = len(order) // 2
        return split(order[:k], parts // 2) + split(order[k:], parts // 2)
    return split(np.arange(len(p)), NCHUNK)


_POISON = np.zeros((3, H))
_POISON[2, : H // 2] = BIG
_POISON[2, H // 2:] = -BIG


def _plan_and_pack(pc, ph, med, ang, tr, cm, hm):
    """Returns (T_prog, in_maps): per-core packed inputs."""
    med64 = med.astype(np.float64)
    ang64 = ang.astype(np.float64)
    tr64 = tr.astype(np.float64)
    G, hulT, degen = _host_coeffs(ph.astype(np.float64), med64, ang64, tr64, hm)
    ptsT = _transform64(pc.astype(np.float64), med64, ang64, tr64)  # (C,N,2)
    hull_ok = hm.sum(-1) >= 3

    hmin = hulT.min(1) - MARGIN
    hmax = hulT.max(1) + MARGIN

    # tiles: (cluster i, point-index array (128,), list of <=SLOTS j's)
    tiles = []
    for i in range(C):
        for ch in _kd_chunks(ptsT[i]):
            q = ptsT[i][ch]
            qmin, qmax = q.min(0), q.max(0)
            # degenerate hulls (no valid edges) contribute viol=1 for every
            # point regardless of position -- never prune them
            alive = [j for j in range(C)
                     if j != i and hull_ok[j]
                     and (degen[j]
                          or ((qmax >= hmin[j]).all()
                              and (hmax[j] >= qmin).all()))]
            for k in range(0, len(alive), SLOTS):
                tiles.append((i, ch, alive[k:k + SLOTS]))

    # distribute: largest-first round-robin is unnecessary (all tiles equal
    # cost) -- just deal them out evenly
    percore = [tiles[c::NCORES] for c in range(NCORES)]
    T = max(1, max(len(p) for p in percore))
    T_prog = ((T + GRP - 1) // GRP) * GRP

    in_maps = []
    for c in range(NCORES):
        pts3 = np.zeros((3, T_prog * PCHUNK), np.float32)
        pts3[2] = 1.0
        gco = np.empty((3, T_prog * TW), np.float64)
        gco[:] = _POISON[:, None, :].repeat(T_prog * SLOTS, 1).reshape(3, -1)
        cm3 = np.zeros((PCHUNK, T_prog * SLOTS), np.float32)
        for t, (i, ch, js) in enumerate(percore[c]):
            pts3[0, t * PCHUNK:(t + 1) * PCHUNK] = pc[i, ch, 0]
            pts3[1, t * PCHUNK:(t + 1) * PCHUNK] = pc[i, ch, 1]
            for s_, j in enumerate(js):
                gco[:, t * TW + s_ * H: t * TW + (s_ + 1) * H] = G[i, :, j, :]
                cm3[:, t * SLOTS + s_] = cm[i, ch]
        in_maps.append({
            "pts3": np.ascontiguousarray(pts3),
            "gcoef": np.ascontiguousarray(gco.astype(np.float32)),
            "cmask": np.ascontiguousarray(cm3),
        })
    return T_prog, in_maps


def _build_nc(T_prog, reps=1):
    import concourse.bacc as bacc
    import concourse.mybir as mybir
    from concourse.tile import TileContext

    f32 = mybir.dt.float32
    nc = bacc.Bacc()

    pts_d = nc.dram_tensor("pts3", [3, T_prog * PCHUNK], f32, kind="ExternalInput")
    g_d = nc.dram_tensor("gcoef", [3, T_prog * TW], f32, kind="ExternalInput")
    cm_d = nc.dram_tensor("cmask", [PCHUNK, T_prog * SLOTS], f32, kind="ExternalInput")
    out_d = nc.dram_tensor("out", [1, 1], f32, kind="ExternalOutput")

    NG = T_prog // GRP
    GW = GRP * SLOTS  # strip columns per group (24)

    with TileContext(nc) as tc:
        with tc.tile_pool(name="const", bufs=1) as cpool, \
             tc.tile_pool(name="work", bufs=4) as wpool, \
             tc.tile_pool(name="psum", bufs=3, space="PSUM") as ppool, \
             tc.tile_pool(name="opsum", bufs=1, space="PSUM") as opool:

            sp = mybir.EngineType.SP
            pts_sb = cpool.tile_from(pts_d[:, :], forced_dma_engine=sp)
            g_sb = cpool.tile_from(g_d[:, :], forced_dma_engine=sp)
            cm_sb = cpool.tile_from(cm_d[:, :], forced_dma_engine=sp)
            vstrip = cpool.tile([PCHUNK, T_prog * SLOTS], f32)
            ones_sb = cpool.tile([PCHUNK, 1], f32)
            nc.vector.memset(ones_sb, 1.0)

            for rep in range(reps):
                for g in range(NG):
                    ps = ppool.tile([PCHUNK, GRP * 128], f32)
                    for k in range(GRP):
                        t = g * GRP + k
                        nc.tensor.matmul(
                            ps[:, k * 128: k * 128 + TW],
                            pts_sb[:, t * PCHUNK:(t + 1) * PCHUNK],
                            g_sb[:, t * TW:(t + 1) * TW],
                            start=True, stop=True,
                        )
                    view = ps.rearrange("p (e r) -> p e r", e=GRP)[:, :, 0:TW] \
                             .rearrange("p e (s h) -> p e s h", h=H)
                    mn = wpool.tile([PCHUNK, GW], f32)
                    nmx = wpool.tile([PCHUNK, GW], f32)
                    nc.vector.tensor_reduce(
                        out=mn, in_=view, axis=mybir.AxisListType.X,
                        op=mybir.AluOpType.min,
                    )
                    nc.vector.tensor_reduce(
                        out=nmx, in_=view, axis=mybir.AxisListType.X,
                        op=mybir.AluOpType.max, negate=True,
                    )
                    v = wpool.tile([PCHUNK, GW], f32)
                    nc.vector.tensor_tensor(
                        out=v, in0=mn, in1=nmx, op=mybir.AluOpType.max)
                    w = wpool.tile([PCHUNK, GW], f32)
                    nc.scalar.activation(
                        out=w, in_=v, func=mybir.ActivationFunctionType.Sigmoid)
                    g01 = wpool.tile([PCHUNK, GW], f32)
                    nc.vector.tensor_scalar(
                        out=g01, in0=v, scalar1=-float(EPS), scalar2=None,
                        op0=mybir.AluOpType.is_ge)
                    q = wpool.tile([PCHUNK, GW], f32)
                    nc.vector.tensor_scalar(
                        out=q, in0=w, scalar1=0.5, scalar2=None,
                        op0=mybir.AluOpType.max)
                    qq = wpool.tile([PCHUNK, GW], f32)
                    nc.vector.tensor_tensor(
                        out=qq, in0=q, in1=g01, op=mybir.AluOpType.mult)
                    nc.vector.tensor_tensor(
                        out=vstrip[:, g * GW:(g + 1) * GW], in0=qq,
                        in1=cm_sb[:, g * GW:(g + 1) * GW],
                        op=mybir.AluOpType.mult)

            acc = cpool.tile([PCHUNK, 1], f32)
            nc.vector.tensor_reduce(
                out=acc, in_=vstrip, axis=mybir.AxisListType.X,
                op=mybir.AluOpType.add,
            )
            out_ps = opool.tile([1, 1], f32)
            nc.tensor.matmul(out_ps, acc, ones_sb, start=True, stop=True)
            out_sb = cpool.tile([1, 1], f32)
            nc.scalar.copy(out=out_sb, in_=out_ps)
            nc.sync.dma_start(out=out_d[:, :], in_=out_sb)

    nc.compile()  # Bacc passes: wait legalization, reg alloc, nop fusion
    return nc


def kernel(padded_clusters, padded_hulls, medoids, rotation_angles,
           translations, cluster_masks, hull_masks):
    pc = np.asarray(padded_clusters, dtype=np.float32)
    ph = np.asarray(padded_hulls, dtype=np.float32)
    med = np.asarray(medoids, dtype=np.float32)
    ang = np.asarray(rotation_angles, dtype=np.float32)
    tr = np.asarray(translations, dtype=np.float32)
    cm = np.asarray(cluster_masks)
    hm = np.asarray(hull_masks)

    T_prog, in_maps = _plan_and_pack(pc, ph, med, ang, tr, cm, hm)

    key = ("nc", T_prog)
    if key not in _NC_CACHE:
        _NC_CACHE[key] = _build_nc(T_prog)
    nc = _NC_CACHE[key]

    from concourse.bass_utils import run_bass_kernel_spmd
    res = run_bass_kernel_spmd(nc, in_maps, core_ids=list(range(NCORES)))
    _NC_CACHE["last_results"] = res

    sep = sum(float(r["out"][0, 0]) for r in res.results)
    total = (SEP_W * sep
             + T_PEN * float(np.sum(tr.astype(np.float64) ** 2))
             + R_PEN * float(np.sum(ang.astype(np.float64) ** 2)))
    return np.asarray(total, dtype=np.float32)
